# revision 6
# baseline (speedup 1.0000x reference)
"""InfoNCE loss kernel for Trainium2, 8 NeuronCores — symmetric version.

sim = Z Z^T is symmetric, so only the (block) upper triangle is computed:
each 512x512 block E = exp(2 z_a z_b^T) contributes its row sums to rows of
group a and its column sums to rows of group b. 16 row-groups of 512 give
136 unique blocks; each core computes 18 blocks (2 diagonal + 16 off-diag;
the 8 antipodal d=8 blocks are computed twice globally and the host drops
the duplicates). This halves both matmul and exp element counts vs the
row-sharded version.

Core k's slots s=0..9 hold groups (2k+s) mod 16. Pairs (lhs slot l, rhs
slot r): l=0: r=0..8; l=1: r=1..9. Pairs are processed in chunks of <=2
sharing the lhs row block so one exp ACTIVATE covers [128, 1024] of PSUM
(2 banks); its accum_out yields the row-sum partials. Column sums are
ones^T @ exp-tile DoubleRow matmuls, packed 4 pairs per PSUM bank via
tile_position col-groups, drained by one DVE copy per 4 pairs; the host
extracts partition rows 0/32/64/96.

Matmuls run fp8 e4m3 with perf_mode=DoubleRow (256-deep contraction).
Embeddings are scaled by ALPHA=16 during on-device normalization so
|z|~0.7 sits in e4m3's normal range; the exp scale compensates with
2/ALPHA^2. Positives reuse the norm chain's rsqrt broadcasts: PE
transposes of rb slices give per-row 1/|x| factors, so no separate
sum-of-squares pass. All activations are forced onto the single table set
containing exp+ln, avoiding ACT_TABLE_LOAD thrash.
"""

import math

import numpy as np
import ml_dtypes

B = 4096
D = 512
N = 2 * B
NCORES = 8
P = 128
ITILES = 8           # own-row subtiles of 128 (2 groups x 4)
CTILES = D // P      # 4
NT = 512
NG = 16              # row groups
GS = N // NG         # 512 rows per group
NSLOTS = 10
INV_T = 2.0
ALPHA = 16.0
LN_ALPHA = math.log(ALPHA)
EXP_SCALE = INV_T / (ALPHA * ALPHA)

PAIRS = [(0, r) for r in range(9)] + [(1, r) for r in range(1, 10)]
# execution chunks: (lhs slot, rhs slots, accum column q)
CHUNKS = [
    (0, (0, 1), 0), (1, (1, 2), 0),
    (0, (2, 3), 1), (1, (3, 4), 1),
    (0, (4, 5), 2), (1, (5, 6), 2),
    (0, (6, 7), 3), (1, (7, 8), 3),
    (0, (8,), 4), (1, (9,), 4),
]
NQ = 5
NCSG = (len(PAIRS) + 3) // 4  # colsum groups of 4 pairs per PSUM bank
# pairs in chunk processing order; colsum bank slot = processing index
CS_ORDER = [(l, r) for l, rs_, q in CHUNKS for r in rs_]
CS_SLOT = {PAIRS.index(p): (i // 4, i % 4) for i, p in enumerate(CS_ORDER)}

_CACHE = {}


def _patch_activation_tables():
    """Force every activation onto the natural_log_exp_and_others set:
    the greedy per-instruction table choice alternates sets for Exp/Ln and
    thrashes ACT_TABLE_LOAD (~1.3us each). Blanking other sets (list order,
    hence act_func_set_id, preserved) hoists a single load."""
    import functools
    from concourse import hw_specs, bacc, bass_interp

    if getattr(hw_specs.get_activation_tables, "_infonce_patched", False):
        return
    orig = hw_specs.get_activation_tables
    KEEP = "natural_log_exp_and_others"

    @functools.cache
    def patched(module_arch):
        tabs = orig(module_arch)
        return {k: (v if k == KEEP else set()) for k, v in tabs.items()}

    patched._infonce_patched = True
    hw_specs.get_activation_tables = patched
    bacc.get_activation_tables = patched
    bass_interp.get_activation_tables = patched


def _build_bass():
    import concourse.bass as bass
    import concourse.tile as tile
    from concourse import bacc, mybir

    _patch_activation_tables()

    dt = mybir.dt
    AF = mybir.ActivationFunctionType
    ALU = mybir.AluOpType
    DR = mybir.MatmulPerfMode.DoubleRow

    nc = bacc.Bacc(None, target_bir_lowering=False, debug=False, num_swdge_queues=4)

    # -------- DRAM I/O --------
    # zt: slot s = raw bf16 embeddings of group (2k+s)%16, d-major:
    # zt[s][p][c][j] = x[c*128+p, group_row j]
    zt_d = nc.dram_tensor("zt", [NSLOTS, P, CTILES, NT], dt.bfloat16,
                          kind="ExternalInput")
    xmT_d = nc.dram_tensor("xmT", [P, ITILES, D], dt.bfloat16, kind="ExternalInput")
    xpT_d = nc.dram_tensor("xpT", [P, ITILES, D], dt.bfloat16, kind="ExternalInput")
    eye_d = nc.dram_tensor("eye", [P, P], dt.bfloat16, kind="ExternalInput")

    rs_d = nc.dram_tensor("rowsums", [P, 2, 4, NQ], dt.float32, kind="ExternalOutput")
    cs_d = nc.dram_tensor("colsums", [1, len(PAIRS), NT], dt.float32,
                          kind="ExternalOutput")
    pt_d = nc.dram_tensor("pt", [P, ITILES], dt.float32, kind="ExternalOutput")

    from contextlib import ExitStack

    with tile.TileContext(nc) as tc, ExitStack() as ctx:
        const = ctx.enter_context(tc.tile_pool(name="const", bufs=1))
        prol = ctx.enter_context(tc.tile_pool(name="prol", bufs=1))
        persist = ctx.enter_context(tc.tile_pool(name="persist", bufs=1))
        nstream = ctx.enter_context(tc.tile_pool(name="nstream", bufs=3))
        small = ctx.enter_context(tc.tile_pool(name="small", bufs=3))
        rbkeep = ctx.enter_context(tc.tile_pool(name="rbkeep", bufs=4))
        ejpool = ctx.enter_context(tc.tile_pool(name="ejp", bufs=2))
        psum_m = ctx.enter_context(tc.tile_pool(name="psum_m", bufs=2, space="PSUM"))
        psum_c = ctx.enter_context(tc.tile_pool(name="psum_c", bufs=1, space="PSUM"))
        psum_s = ctx.enter_context(tc.tile_pool(name="psum_s", bufs=1, space="PSUM"))
        psum_b = ctx.enter_context(tc.tile_pool(name="psum_b", bufs=1, space="PSUM"))

        ones8dr = const.tile([P, 2, 16], dt.float8e4)
        nc.vector.memset(ones8dr, 1.0)
        ones_colb = const.tile([P, 1], dt.bfloat16)
        nc.vector.memset(ones_colb, 1.0)
        ones_row = const.tile([1, P], dt.bfloat16)
        nc.vector.memset(ones_row, 1.0)
        lnalpha = const.tile([P, 1], dt.float32)
        nc.vector.memset(lnalpha, LN_ALPHA)

        cs_stage = persist.tile([1, len(PAIRS), NT], dt.float32)
        zt_f = persist.tile([P, NSLOTS, CTILES, NT], dt.bfloat16)
        zs_f = persist.tile([P, NSLOTS, CTILES, NT], dt.float8e4)
        rowpart = persist.tile([P, 2, 4, NQ], dt.float32)
        eye_s = const.tile([P, P], dt.bfloat16)

        # all input DMA descriptors up front
        for s in range(NSLOTS):
            nc.gpsimd.dma_start(out=zt_f[:, s], in_=zt_d[s])
        xmT_s = prol.tile([P, ITILES, D], dt.bfloat16)
        nc.gpsimd.dma_start(out=xmT_s, in_=xmT_d[:])
        xpT_s = prol.tile([P, ITILES, D], dt.bfloat16)
        nc.gpsimd.dma_start(out=xpT_s, in_=xpT_d[:])
        nc.gpsimd.dma_start(out=eye_s, in_=eye_d[:])

        # positives side tiles (filled by rb transposes in the norm chains)
        zmT = prol.tile([P, ITILES, D], dt.bfloat16)
        zpT = prol.tile([P, ITILES, D], dt.bfloat16)
        tjunk = prol.tile([P, ITILES, D], dt.bfloat16)
        posT = small.tile([P, ITILES], dt.float32)

        def norm_slot(s):
            sq = nstream.tile([P, CTILES, NT], dt.bfloat16, name=f"sq_{s}",
                              tag="sq", bufs=3)
            nc.vector.tensor_mul(sq, zt_f[:, s], zt_f[:, s])
            ps = psum_s.tile([1, NT], dt.float32, name=f"ps_{s}", tag="ps")
            for c in range(CTILES):
                nc.tensor.matmul(ps, ones_colb, sq[:, c, :],
                                 start=(c == 0), stop=(c == CTILES - 1))
            ln_n = small.tile([1, NT], dt.bfloat16, name=f"ln_{s}", tag="ln")
            nc.scalar.activation(ln_n, ps, AF.Ln)
            pb = psum_b.tile([P, NT], dt.float32, name=f"pb_{s}", tag="pb")
            nc.tensor.matmul(pb, ones_row, ln_n)
            rb = rbkeep.tile([P, NT], dt.bfloat16, name=f"rb_{s}", tag="rb",
                             bufs=4)
            nc.scalar.activation(rb, pb, AF.Exp, scale=-0.5, bias=lnalpha)
            rb_b = bass.AP(tensor=rb.tensor, offset=rb.offset,
                           ap=[rb.ap[0], [0, CTILES], rb.ap[1]])
            nc.vector.tensor_mul(zs_f[:, s], zt_f[:, s], rb_b)
            # positives prep: slots 0/1 are the own row groups, 8/9 the
            # partner groups; rb holds ALPHA/|x_row| broadcast across
            # partitions -- a PE transpose turns columns into partitions.
            if s in (0, 1, 8, 9):
                dstT, xT = (zmT, xmT_s) if s in (0, 1) else (zpT, xpT_s)
                for ii in range(4):
                    i = (s % 2) * 4 + ii
                    rbt = psum_b.tile([P, P], dt.bfloat16,
                                      name=f"rbt_{s}_{ii}", tag="rbt")
                    nc.tensor.transpose(rbt, rb[:, ii * P:(ii + 1) * P], eye_s)
                    rcol = small.tile([P, 1], dt.float32,
                                      name=f"rcol_{s}_{ii}", tag="rcol")
                    nc.vector.tensor_copy(rcol, rbt[:, 0:1])
                    nc.vector.tensor_scalar_mul(dstT[:, i, :], xT[:, i, :],
                                                rcol)

        for s in range(4):
            norm_slot(s)
        next_norm = 4

        def _ts_pos():
            nc.vector.tensor_mul(tjunk, zmT, zpT)

        def _ts_red():
            nc.vector.tensor_reduce(posT, tjunk, axis=mybir.AxisListType.X,
                                    op=ALU.add)

        TSTEPS = [_ts_pos, _ts_red]

        for ci, (l, rs_, q) in enumerate(CHUNKS):
            W = len(rs_)
            ej = ejpool.tile([P, 4, 2, NT], dt.float8e4, name=f"ej_{l}_{q}",
                             tag="ej", bufs=2)
            for ii in range(4):
                pm = psum_m.tile([P, 2, NT], dt.float32,
                                 name=f"pm_{l}_{q}_{ii}", tag="pm")
                for t, r in enumerate(rs_):
                    for cc in range(CTILES // 2):
                        nc.tensor.matmul(
                            pm[:, t, :],
                            zs_f[:, l, 2 * cc:2 * cc + 2, ii * P:(ii + 1) * P],
                            zs_f[:, r, 2 * cc:2 * cc + 2, :],
                            start=(cc == 0), stop=(cc == CTILES // 2 - 1),
                            perf_mode=DR)
                nc.scalar.activation(ej[:, ii, :W, :], pm[:, :W, :], AF.Exp,
                                     scale=EXP_SCALE,
                                     accum_out=rowpart[:, l, ii, q:q + 1])
            def emit_colsums(l=l, rs_=rs_, ej=ej):
                for t, r in enumerate(rs_):
                    pidx = PAIRS.index((l, r))
                    pc = psum_c.tile([1, NT], dt.float32, name=f"pc_{l}_{r}",
                                     tag="pc")
                    for iip in range(2):
                        nc.tensor.matmul(pc, ones8dr[:, :, 0:1],
                                         ej[:, 2 * iip:2 * iip + 2, t, :],
                                         start=(iip == 0), stop=(iip == 1),
                                         perf_mode=DR)
                    nc.vector.tensor_copy(cs_stage[:, pidx, :], pc)
            emit_colsums()
            if next_norm < NSLOTS:
                norm_slot(next_norm)
                next_norm += 1
            elif TSTEPS:
                TSTEPS.pop(0)()

        while TSTEPS:
            TSTEPS.pop(0)()

        nc.gpsimd.dma_start(out=pt_d[:], in_=posT)
        nc.gpsimd.dma_start(out=rs_d[:], in_=rowpart)
        nc.gpsimd.dma_start(out=cs_d[:], in_=cs_stage)

    nc.compile()
    return nc


def _get_nc():
    if "nc" not in _CACHE:
        _CACHE["nc"] = _build_bass()
    return _CACHE["nc"]


def _prep_inputs(polyline_embs, c_embs):
    bf16 = ml_dtypes.bfloat16
    z = np.concatenate([np.asarray(polyline_embs, np.float32),
                        np.asarray(c_embs, np.float32)], axis=0)  # [8192, 512]
    zb = z.astype(bf16)

    xtb = np.ascontiguousarray(zb.T)  # [512, 8192] bf16
    gtiles = []
    for g in range(NG):
        t = xtb[:, g * GS:(g + 1) * GS].reshape(CTILES, P, NT).transpose(1, 0, 2)
        gtiles.append(np.ascontiguousarray(t))  # [128, 4, 512]

    eye = np.eye(P, dtype=bf16)
    in_maps = []
    RPC = N // NCORES
    for k in range(NCORES):
        zt = np.stack([gtiles[(2 * k + s) % NG] for s in range(NSLOTS)])
        rows = zb[k * RPC:(k + 1) * RPC]
        prows_start = (k * RPC + B) % N
        prows = zb[prows_start:prows_start + RPC]
        xmT = np.ascontiguousarray(
            rows.reshape(ITILES, P, D).transpose(1, 0, 2))
        xpT = np.ascontiguousarray(
            prows.reshape(ITILES, P, D).transpose(1, 0, 2))
        in_maps.append({"zt": zt, "xmT": xmT, "xpT": xpT, "eye": eye})
    return in_maps


def _combine(results):
    denom = np.zeros(N, np.float64)
    pos = np.zeros(N, np.float64)
    a2 = float(ALPHA) ** 2
    for k, r in enumerate(results):
        rp = r["rowsums"].astype(np.float64)        # [P, 2, 4, NQ]
        use_q = NQ if k < 4 else NQ - 1             # drop duplicate d=8 chunk
        rsum = rp[:, :, :, :use_q].sum(axis=3)      # [P, 2, 4]
        for l in range(2):
            for ii in range(4):
                base = k * 1024 + l * 512 + ii * 128
                denom[base:base + 128] += rsum[:, l, ii]
        cs = r["colsums"].astype(np.float64)[0]     # [18, NT]
        for idx, (l, rr) in enumerate(PAIRS):
            d = rr - l
            if d == 0 or (d == 8 and k >= 4):
                continue
            g = (2 * k + rr) % NG
            denom[g * GS:(g + 1) * GS] += cs[idx]
        pt = r["pt"].astype(np.float64)             # [P, ITILES], alpha^2-scaled
        for i in range(ITILES):
            base = k * 1024 + i * 128
            pos[base:base + 128] = pt[:, i] / a2
    denom -= np.exp(INV_T)  # self-sim == 1 after normalization
    loss = np.mean(np.log(denom) - INV_T * pos)
    return np.float32(loss), denom, pos


def kernel(polyline_embs, c_embs):
    from concourse.bass_utils import run_bass_kernel_spmd

    nc = _get_nc()
    in_maps = _prep_inputs(polyline_embs, c_embs)
    res = run_bass_kernel_spmd(nc, in_maps, core_ids=list(range(NCORES)))
    _CACHE["last_results"] = res
    loss, denom, pos = _combine(res.results)
    _CACHE["last_denom"] = denom
    _CACHE["last_pos"] = pos
    return loss
